# revision 2
# baseline (speedup 1.0000x reference)
"""Trainium2 Bass kernel for nn_DRuleLoss.

Math (exact collapse of the reference):
    branches = min(H.sum(1), 1)                 # [n]
    bc       = branches.sum()
    rmax     = H.max(1); rmin = H.min(1)        # [n]
    loss = sum_{b,i} [ branches[i]*p + branches[i]*p*max(p*rmax[i], p*rmin[i]) ] / bc
         (p = y_pred[b,i])

For p >= 0 (graded inputs are uniform [0,1)): max(p*rmax, p*rmin) = p*rmax, so
    loss = sum_i w1[i]*colsum_p[i] + sum_i w2[i]*colsum_p2[i]
with w1 = branches/bc, w2 = branches*rmax/bc, colsum_* = batch column sums.
A host-side correction handles any p < 0 exactly (never triggered for graded data).

Device strategy (data-parallel, 8 cores, batch-sharded):
  Each core's y shard [512, 8192] streams in as column slabs shaped
  [128, 4, slab] (batch rows folded into the free dim).  Slab DMAs
  alternate between the two HWDGE queues (SP + Activation) — a single
  queue tops out ~300 GB/s, both together reach ~366 GB/s on real HW.
  Per 512-column chunk: ScalarE squares the chunk (f32 -> f32r),
  TensorE column-sums the chunk and its square via matmuls against a
  ones[128,1] stationary vector (float32r: 1 cycle/row), accumulating
  the 4 row-subtiles into a PSUM bank slot.  A fused DVE
  scalar_tensor_tensor (mult + free-dim sum) dots each finished slot
  with its weight row (all weights on partition 0's free space) into
  res[0, s]; one final DMA writes the 32 per-slot dots.  The host sums
  8 x 32 scalars.
  H never touches the device (only its per-row reductions, folded into w).
"""

import numpy as np

import concourse.tile as tile
import concourse.mybir as mybir
from concourse import bacc
from concourse.bass_utils import run_bass_kernel_spmd

N_CORES = 8
B, N = 4096, 8192
BS = B // N_CORES        # 512 rows per core
T = BS // 128            # 4 row-subtiles folded into the free dim
CH = 512                 # matmul free-dim chunk (one PSUM bank, fp32)
NCHUNK = N // CH         # 16
# chunks per DMA slab; a smaller final slab shortens the post-DMA tail
SLAB_CHUNKS = (4, 4, 3, 2, 1, 1, 1)
F32 = mybir.dt.float32
F32R = mybir.dt.float32r
BF16 = mybir.dt.bfloat16

_NC_CACHE = {}
LAST_RESULTS = None      # BassKernelResults of the most recent device run


def build_pools(tc):
    import contextlib
    st = contextlib.ExitStack()
    pools = {
        "slabs": st.enter_context(tc.tile_pool(name="slabs", bufs=3)),
        "sq": st.enter_context(tc.tile_pool(name="sq", bufs=2)),
        "small": st.enter_context(tc.tile_pool(name="small", bufs=1)),
        "pp": st.enter_context(tc.tile_pool(name="pp", bufs=4)),
        "psum": st.enter_context(tc.tile_pool(name="psum", bufs=8,
                                              space="PSUM")),
    }
    return st, pools


def build_prelude(nc, pools, w):
    """One-time setup: ones vector, weight tile, result tile."""
    small = pools["small"]
    ones_f = small.tile([128, 1], F32)
    nc.vector.memset(ones_f[:], 1.0)
    ones = small.tile([128, 1], F32R)
    nc.vector.tensor_copy(ones[:], ones_f[:])
    # all weights on partition 0 so every TTR operand has base
    # partition 0 (non-zero compute base partitions fail codegen)
    wt = small.tile([1, 2 * NCHUNK * CH], F32)
    # SWDGE queue: keeps the weight load off the two slab HWDGE queues
    nc.gpsimd.dma_start(wt[:], w[:])
    res = small.tile([1, 2 * NCHUNK], F32)
    return ones, wt, res


def build_body(nc, y_v, pools, ones, wt, res):
    """One full pass over the core's [512, 8192] shard."""
    slabs, sq, pp, psum = (pools["slabs"], pools["sq"], pools["pp"],
                           pools["psum"])
    c0 = 0
    for k, nch in enumerate(SLAB_CHUNKS):
        width = nch * CH
        slab = slabs.tile([128, T, max(SLAB_CHUNKS) * CH], F32R,
                          tag="slab", name="slab")
        # alternate the two HWDGE queues; one queue caps ~300 GB/s
        q = nc.sync if k % 2 == 0 else nc.scalar
        q.dma_start(
            slab[:, :, :width],
            y_v[:, :, c0 * CH:c0 * CH + width],
        )
        for cl in range(nch):
            c = c0 + cl
            ysl = slab[:, :, cl * CH:(cl + 1) * CH]
            st = sq.tile([128, T, CH], F32R, tag="st", name="st")
            last = (k == len(SLAB_CHUNKS) - 1 and cl == nch - 1)
            if last:
                # split the final square so its q=1 matmuls overlap
                # the second half instead of waiting for the whole op
                nc.scalar.activation(
                    st[:, 0:2, :], ysl[:, 0:2, :],
                    mybir.ActivationFunctionType.Square)
                nc.scalar.activation(
                    st[:, 2:4, :], ysl[:, 2:4, :],
                    mybir.ActivationFunctionType.Square)
            else:
                nc.scalar.activation(st[:], ysl,
                                     mybir.ActivationFunctionType.Square)
            for q_, src in ((0, ysl), (1, st)):
                s = q_ * NCHUNK + c
                slot = psum.tile([1, CH], F32, tag="slot", name="slot")
                for t in range(T):
                    nc.tensor.matmul(
                        slot[:],
                        ones[:],
                        src[:, t, :],
                        start=(t == 0),
                        stop=(t == T - 1),
                    )
                prod = pp.tile([1, CH], F32, tag="prod", name="prod")
                nc.vector.scalar_tensor_tensor(
                    out=prod[:],
                    in0=slot[:],
                    scalar=1.0,
                    in1=wt[0:1, s * CH:(s + 1) * CH],
                    op0=mybir.AluOpType.mult,
                    op1=mybir.AluOpType.mult,
                    accum_out=res[0:1, s:s + 1],
                )
        c0 += nch


def build_epilogue(nc, out, res):
    # q=0 results finish before the last q=1 STT; ship them early
    nc.sync.dma_start(out[0:1, 0:NCHUNK], res[0:1, 0:NCHUNK])
    nc.sync.dma_start(out[0:1, NCHUNK:], res[0:1, NCHUNK:])


def _build_nc():
    nc = bacc.Bacc("TRN2", target_bir_lowering=False, debug=False,
                   num_devices=N_CORES)
    y = nc.dram_tensor("y", [BS, N], F32R, kind="ExternalInput")
    w = nc.dram_tensor("w", [1, 2 * NCHUNK * CH], F32, kind="ExternalInput")
    out = nc.dram_tensor("out", [1, 2 * NCHUNK], F32, kind="ExternalOutput")

    # y row (t*128 + p) -> partition p, free (t, n)
    y_v = y.rearrange("(t p) n -> p t n", p=128)

    with tile.TileContext(nc) as tc:
        st, pools = build_pools(tc)
        with st:
            ones, wt, res = build_prelude(nc, pools, w)
            build_body(nc, y_v, pools, ones, wt, res)
            build_epilogue(nc, out, res)

    nc.compile()
    return nc


def _get_nc():
    if "nc" not in _NC_CACHE:
        _NC_CACHE["nc"] = _build_nc()
    return _NC_CACHE["nc"]


def _weight_layout(w1, w2):
    """Pack w1/w2 [N] into [1, 32*512]: block s = q*16 + c holds chunk c of wq."""
    W = np.empty((1, 2 * NCHUNK * CH), dtype=np.float32)
    for s in range(2 * NCHUNK):
        q, c = divmod(s, NCHUNK)
        vec = w1 if q == 0 else w2
        W[0, s * CH:(s + 1) * CH] = vec[c * CH:(c + 1) * CH]
    return W


def kernel(y_pred, H, y_true):
    global LAST_RESULTS
    y_pred = np.ascontiguousarray(np.asarray(y_pred, dtype=np.float32))
    H = np.asarray(H, dtype=np.float32)

    branches = np.minimum(H.sum(axis=1, dtype=np.float64), 1.0)
    bc = float(branches.sum())
    rmax = H.max(axis=1).astype(np.float64)
    rmin = H.min(axis=1).astype(np.float64)
    w1 = (branches / bc).astype(np.float32)
    w2a = (branches * rmax / bc).astype(np.float32)
    w2b = (branches * rmin / bc).astype(np.float32)

    # Device assumes max(p*rmax, p*rmin) == p*rmax, true for p >= 0.
    # Exact host correction for any negative p (graded inputs have none).
    corr = 0.0
    if np.any(y_pred < 0):
        neg = np.minimum(y_pred, 0.0).astype(np.float64)
        corr = float(((neg * neg) @ (w2b - w2a).astype(np.float64)).sum())

    W = _weight_layout(w1, w2a)
    nc = _get_nc()
    in_maps = [
        {"y": np.ascontiguousarray(y_pred[i * BS:(i + 1) * BS]), "w": W}
        for i in range(N_CORES)
    ]
    LAST_RESULTS = run_bass_kernel_spmd(nc, in_maps,
                                        core_ids=list(range(N_CORES)))
    total = sum(
        float(r["out"].sum(dtype=np.float64)) for r in LAST_RESULTS.results
    )
    return np.float32(total + corr)


# revision 3
# speedup vs baseline: 1.0566x; 1.0566x over previous
"""Trainium2 Bass kernel for nn_DRuleLoss.

Math (exact collapse of the reference):
    branches = min(H.sum(1), 1)                 # [n]
    bc       = branches.sum()
    rmax     = H.max(1); rmin = H.min(1)        # [n]
    loss = sum_{b,i} [ branches[i]*p + branches[i]*p*max(p*rmax[i], p*rmin[i]) ] / bc
         (p = y_pred[b,i])

For p >= 0 (graded inputs are uniform [0,1)): max(p*rmax, p*rmin) = p*rmax, so
    loss = sum_i w1[i]*colsum_p[i] + sum_i w2a[i]*colsum_p2[i] + neg_corr
with w1 = branches/bc, w2a = branches*rmax/bc.

H is a tree adjacency (one parent per non-root row), so w1 and w2a are
the CONSTANT 1/bc on every column except a handful of deviants (just
column 0 for the root).  The device therefore computes only the
unweighted scalar  S = sum_{b,i} (p + p^2)  per core; the host forms
    loss = alpha*S_total + sum_{i in D} [(w1[i]-alpha)*colsum_p[i]
                                         + (w2a[i]-alpha)*colsum_p2[i]]
           + sum_i (w2b[i]-w2a[i]) * negsum2[i]
where alpha is the modal weight, D the deviant columns (exact numpy on
the few y_pred[:, D] columns), and the last term the exact correction
for negative p (empty for graded data).  Fully general for any H.

Device strategy (data-parallel, 8 cores, batch-sharded):
  Each core's y shard [512, 8192] streams in as column slabs shaped
  [128, 4, slab] (batch rows folded into the free dim).  Slab DMAs
  split across BOTH HWDGE queues (SP + Activation; one queue caps
  ~300 GB/s, together ~366 GB/s) and each config is issued 2 slabs
  ahead of its compute so neither sequencer stalls a ring.  Per
  512-column chunk: square on ScalarE or DVE (split so the Act stream
  stays short), then TensorE accumulates BOTH the chunk and its square
  via ones[128,1]-stationary matmuls (float32r: 1 cycle/row) into one
  of two PSUM slots (chunks 0-7 -> slot 0, 8-15 -> slot 1, 64-matmul
  accumulation groups).  One DVE scalar_tensor_tensor per slot dots it
  with a ones row into res[0, h].  One final DMA ships res [1, 2];
  the host sums 8 x 2 scalars.  H never touches the device.
"""

import numpy as np

import concourse.tile as tile
import concourse.mybir as mybir
from concourse import bacc
from concourse.bass_utils import run_bass_kernel_spmd

N_CORES = 8
B, N = 4096, 8192
BS = B // N_CORES        # 512 rows per core
T = BS // 128            # 4 row-subtiles folded into the free dim
CH = 512                 # matmul free-dim chunk (one PSUM bank, fp32)
NCHUNK = N // CH         # 16
# chunks per DMA slab; small final slabs shorten the post-DMA tail
SLAB_CHUNKS = (3, 3, 3, 3, 2, 1, 1)
# 0 = sync (SP) queue, 1 = scalar (Activation) queue; 8 chunks each
SLAB_QUEUE = (0, 1, 0, 1, 1, 0, 0)
# squares on ScalarE only for slabs 0 and 2: keeps the Act instruction
# stream short so its DMA configs execute early; the rest go to DVE
SQ_ON_ACT = (True, False, True, False, False, False, False)
AHEAD = 2                # slabs of DMA run-ahead in program order
HALF = NCHUNK // 2       # chunks per PSUM accumulation slot
F32 = mybir.dt.float32
F32R = mybir.dt.float32r
BF16 = mybir.dt.bfloat16

_NC_CACHE = {}
LAST_RESULTS = None      # BassKernelResults of the most recent device run


def build_pools(tc):
    import contextlib
    st = contextlib.ExitStack()
    pools = {
        "slabs": st.enter_context(tc.tile_pool(name="slabs", bufs=4)),
        "sq": st.enter_context(tc.tile_pool(name="sq", bufs=3)),
        "small": st.enter_context(tc.tile_pool(name="small", bufs=1)),
        "pp": st.enter_context(tc.tile_pool(name="pp", bufs=2)),
        "psum": st.enter_context(tc.tile_pool(name="psum", bufs=4,
                                              space="PSUM")),
    }
    return st, pools


def build_prelude(nc, pools):
    """One-time setup: ones column (matmul stationary), ones row (final
    dot), result tile."""
    small = pools["small"]
    ones_f = small.tile([128, 1], F32)
    nc.vector.memset(ones_f[:], 1.0)
    ones = small.tile([128, 1], F32R)
    nc.vector.tensor_copy(ones[:], ones_f[:])
    ones_row = small.tile([1, CH], F32)
    nc.vector.memset(ones_row[:], 1.0)
    res = small.tile([1, 2], F32)
    return ones, ones_row, res


def build_body(nc, y_v, pools, ones, ones_row, res):
    """One full pass over the core's [512, 8192] shard."""
    slabs, sq, pp, psum = (pools["slabs"], pools["sq"], pools["pp"],
                           pools["psum"])
    nslab = len(SLAB_CHUNKS)
    offs = [0]
    for nch in SLAB_CHUNKS:
        offs.append(offs[-1] + nch)
    width_max = max(SLAB_CHUNKS) * CH

    slab_tiles = {}

    def issue(k):
        width = SLAB_CHUNKS[k] * CH
        tl = slabs.tile([128, T, width_max], F32R, tag="slab", name="slab")
        q = nc.sync if SLAB_QUEUE[k] == 0 else nc.scalar
        q.dma_start(tl[:, :, :width],
                    y_v[:, :, offs[k] * CH:offs[k] * CH + width])
        slab_tiles[k] = tl

    issue(0)
    issue(1)
    slot = None
    for k in range(nslab):
        if k + AHEAD < nslab:
            issue(k + AHEAD)
        slab = slab_tiles.pop(k)
        for cl in range(SLAB_CHUNKS[k]):
            c = offs[k] + cl
            h = c // HALF
            ysl = slab[:, :, cl * CH:(cl + 1) * CH]
            st = sq.tile([128, T, CH], F32R, tag="st", name="st")
            if SQ_ON_ACT[k]:
                nc.scalar.activation(st[:], ysl,
                                     mybir.ActivationFunctionType.Square)
            else:
                nc.vector.scalar_tensor_tensor(
                    out=st[:], in0=ysl, scalar=1.0, in1=ysl,
                    op0=mybir.AluOpType.mult, op1=mybir.AluOpType.mult)
            if c % HALF == 0:
                slot = psum.tile([1, CH], F32, tag="slot", name="slot")
            for q_, src in ((0, ysl), (1, st)):
                for t in range(T):
                    nc.tensor.matmul(
                        slot[:],
                        ones[:],
                        src[:, t, :],
                        start=(c % HALF == 0 and q_ == 0 and t == 0),
                        stop=(c % HALF == HALF - 1 and q_ == 1
                              and t == T - 1),
                    )
            if c % HALF == HALF - 1:
                prod = pp.tile([1, CH], F32, tag="prod", name="prod")
                nc.vector.scalar_tensor_tensor(
                    out=prod[:],
                    in0=slot[:],
                    scalar=1.0,
                    in1=ones_row[:],
                    op0=mybir.AluOpType.mult,
                    op1=mybir.AluOpType.mult,
                    accum_out=res[0:1, h:h + 1],
                )


def build_epilogue(nc, out, res):
    nc.sync.dma_start(out[:], res[:])


def _build_nc():
    nc = bacc.Bacc("TRN2", target_bir_lowering=False, debug=False,
                   num_devices=N_CORES)
    y = nc.dram_tensor("y", [BS, N], F32R, kind="ExternalInput")
    out = nc.dram_tensor("out", [1, 2], F32, kind="ExternalOutput")

    # y row (t*128 + p) -> partition p, free (t, n)
    y_v = y.rearrange("(t p) n -> p t n", p=128)

    with tile.TileContext(nc) as tc:
        st, pools = build_pools(tc)
        with st:
            ones, ones_row, res = build_prelude(nc, pools)
            build_body(nc, y_v, pools, ones, ones_row, res)
            build_epilogue(nc, out, res)

    nc.compile()
    return nc


def _get_nc():
    if "nc" not in _NC_CACHE:
        _NC_CACHE["nc"] = _build_nc()
    return _NC_CACHE["nc"]


def kernel(y_pred, H, y_true):
    global LAST_RESULTS
    y_pred = np.ascontiguousarray(np.asarray(y_pred, dtype=np.float32))
    H = np.asarray(H, dtype=np.float32)

    branches = np.minimum(H.sum(axis=1, dtype=np.float64), 1.0)
    bc = float(branches.sum())
    rmax = H.max(axis=1).astype(np.float64)
    rmin = H.min(axis=1).astype(np.float64)
    w1 = (branches / bc).astype(np.float32)
    w2a = (branches * rmax / bc).astype(np.float32)
    w2b = (branches * rmin / bc).astype(np.float32)

    # modal weight: device computes the unweighted sum, host rescales
    vals, counts = np.unique(w1, return_counts=True)
    alpha = float(vals[np.argmax(counts)])
    dev = (w1 != np.float32(alpha)) | (w2a != np.float32(alpha))
    D = np.nonzero(dev)[0]

    corr = 0.0
    if D.size:
        yd = y_pred[:, D].astype(np.float64)
        cp = yd.sum(axis=0)
        cp2 = (yd * yd).sum(axis=0)
        corr += float(((w1[D].astype(np.float64) - alpha) * cp).sum()
                      + ((w2a[D].astype(np.float64) - alpha) * cp2).sum())

    # Device assumes max(p*rmax, p*rmin) == p*rmax, true for p >= 0.
    # Exact correction for any negative p (graded inputs have none).
    if np.any(y_pred < 0):
        neg = np.minimum(y_pred, 0.0).astype(np.float64)
        corr += float(((neg * neg) @ (w2b - w2a).astype(np.float64)).sum())

    nc = _get_nc()
    in_maps = [
        {"y": np.ascontiguousarray(y_pred[i * BS:(i + 1) * BS])}
        for i in range(N_CORES)
    ]
    LAST_RESULTS = run_bass_kernel_spmd(nc, in_maps,
                                        core_ids=list(range(N_CORES)))
    total = sum(
        float(r["out"].sum(dtype=np.float64)) for r in LAST_RESULTS.results
    )
    return np.float32(alpha * total + corr)
